# revision 25
# baseline (speedup 1.0000x reference)
"""BitLinear forward kernel for Trainium2 (8 NeuronCores, data-parallel),
fp8 DoubleRow edition, v4 schedule.

Forward math of the reference (straight-through estimators resolved):
    out = (x_quant/scale) @ w_q^T
    x_int = round(x_norm * 127/amax_norm) = round(x * 127/amax)   (rms cancels)
    x_quant/scale = x_int * amax/(127*rms)
    w_q = (w > 0.5*(gamma+eps)) in {0,1}     (w >= 0 here)

Device scheme (per core, 2048 tokens, 16 token tiles of 128):
  * complement weights Wc = 1 - w_q = (w <= thr): density ~0.25, so the fp8
    quantization error of x only flows through 1/4 of the terms:
        out = (S - x_int @ Wc) * os
  * x loaded as fp16 via gpsimd casting DMA (halves the x HBM charge; adds
    ~0.1% relative error, budget-checked).  x_int via the magic-constant RNE
    trick; S[t] fused into the quantize pass (accum_out).
  * x_int -> fp8 via PE transpose + ACT copy for early tiles; via the XBAR
    DMA-transpose for late tiles (the DMA engines are otherwise idle there,
    and it unloads the PE which paces the back half).
  * matmuls fp8 MatmulPerfMode.DoubleRow; per-token scale os =
    1/sqrt(ssq_int/2048) from the DoubleRow gram diagonal.
  * gamma = mean|W| distributed: host rolls each core's wT (and x's feature
    dim identically) so chunks 0,1 of the core's stream are its private
    256-row slice; local slice sum -> tiny AllGather (15us vs 28us for
    AllReduce) -> total.  No separate wg input.

v4 schedule:
  * W owns the DMA early: w0..w15 back-to-back on the SP queue (collective
    bounce DMAs jump the queue via dep edges), so all 8 quantized weight
    pairs exist by ~52us and the PSUM accumulation never stalls.
  * x tiles 0-2 load during the W phase (casting DMAs on the Pool queue);
    tiles 1-2's y ops run inside the 15us collective window.
  * Weight quant split DVE/Pool; engine loadout per tile in the crunch:
    DVE amax+xq+1 drain, ACT 2 casts + 2 drains, Pool y+dsc+1 drain,
    PE gram+mms (+transposes for early tiles only).
"""
import numpy as np

import concourse.bass as bass
import concourse.bacc as bacc
import concourse.bass_isa as bass_isa
import concourse.mybir as mybir
import concourse.tile as tile
from concourse.bass_utils import run_bass_kernel_spmd
from concourse.masks import make_identity

F32 = mybir.dt.float32
FP16 = mybir.dt.float16
BF16 = mybir.dt.bfloat16
FP8 = mybir.dt.float8e4
DR = mybir.MatmulPerfMode.DoubleRow
AF = mybir.ActivationFunctionType
ALU = mybir.AluOpType

NCORES = 8
B, S, DIN, DOUT = 4, 4096, 2048, 2048
T = (B * S) // NCORES        # tokens per core = 2048
TP = T // 128                # token tiles per core = 16
KC = DIN // 128              # contraction chunks = 16
NP = KC // 2                 # DoubleRow k-pairs = 8
KH = KC // 2                 # chunks per transpose half = 8
QW = DOUT // 4               # psum quarter width = 512

C_MAGIC = 12582912.0         # 1.5 * 2**23: fp32 round-to-nearest-even trick
EPS_GAMMA = 1e-5

# which engine quantizes W chunk j (DVE is 2x-fast; it gets the first and
# last blocks so the last pair is ready ~1.1us after the last W chunk lands)
QUANT_ENG = {0: "dve", 1: "dve", 2: "dve", 3: "dve", 4: "dve", 5: "dve",
             6: "pool", 7: "pool", 8: "pool", 9: "pool", 10: "pool",
             11: "pool", 12: "dve", 13: "dve", 14: "dve", 15: "dve"}
# pair consumption order for the pair-streamed tile 0 (by readiness)
PAIR_ORDER = [0, 1, 3, 2, 4, 6, 5, 7]
# drain engine per psum quarter
DRAIN_ENG = ["act", "act", "dve", "pool"]
# tiles whose transpose goes through the XBAR DMA-transpose unit
XBAR_TILES = frozenset(range(10, TP))


class Ctx:
    pass


def _emit_chain_front(nc, cx, i):
    """Tile i: amax/m/y/xq/transpose/cast/gram.  The scale finish (dsc, os)
    is emitted separately one iteration later (see _emit_finish)."""
    st = cx.st
    xf = cx.xf[i]

    amax = st.tile([128, 1], F32, tag="amax", name=f"amax{i}")
    nc.vector.tensor_reduce(out=amax[:], in_=xf[:], axis=mybir.AxisListType.X,
                            op=ALU.max, apply_absolute_value=True)
    # m = 127/amax, Newton-refined (tiny [128,1] ops are ~free)
    rcp = st.tile([128, 1], F32, tag="rcp", name=f"rcp{i}")
    nc.vector.reciprocal(rcp[:], amax[:])
    t0 = st.tile([128, 1], F32, tag="t0", name=f"t0_{i}")
    nc.vector.tensor_mul(t0[:], amax[:], rcp[:])
    u0 = st.tile([128, 1], F32, tag="u0", name=f"u0_{i}")
    nc.vector.tensor_scalar(out=u0[:], in0=t0[:], scalar1=2.0, scalar2=-1.0,
                            op0=ALU.subtract, op1=ALU.mult)
    rcp1 = st.tile([128, 1], F32, tag="rcp1", name=f"rcp1_{i}")
    nc.vector.tensor_mul(rcp1[:], rcp[:], u0[:])
    m = st.tile([128, 1], F32, tag="m", name=f"m{i}")
    nc.vector.tensor_scalar_mul(m[:], rcp1[:], 127.0)

    _emit_chain_y(nc, cx, i, m)
    _emit_chain_tail(nc, cx, i)


def _emit_chain_y(nc, cx, i, m):
    # y = x*m + C on Pool
    y = cx.yp.tile([128, DIN], F32, tag="y", name=f"y{i}")
    nc.gpsimd.tensor_scalar(out=y[:], in0=cx.xf[i][:], scalar1=m[:],
                            scalar2=C_MAGIC, op0=ALU.mult, op1=ALU.add)
    cx.y[i] = y


def _emit_chain_tail(nc, cx, i):
    st = cx.st
    # xq = y - C -> bf16 ints, S = sum_d xq (DVE 2x + free accum)
    xq = cx.xqp.tile([128, DIN], BF16, tag="xq", name=f"xq{i}")
    S_col = st.tile([128, 1], F32, tag="S", name=f"S{i}")
    nc.vector.tensor_scalar(out=xq[:], in0=cx.y[i][:],
                            scalar1=C_MAGIC, scalar2=0.0,
                            op0=ALU.subtract, op1=ALU.add,
                            accum_out=S_col[:])
    cx.S[i] = S_col

    xqT = cx.xqTp.tile([128, KC, 128], FP8, tag="xqT", name=f"xqT{i}")
    if i in XBAR_TILES:
        # XBAR DMA-transpose (SP queue) -> bf16 SBUF, one ACT cast -> fp8
        xqTb = cx.xqTbp.tile([128, KC, 128], BF16, tag="xqTb",
                             name=f"xqTb{i}")
        nc.sync.dma_start_transpose(xqTb[:, :, :], xq[:, :])
        nc.scalar.activation(out=xqT[:, :, :], in_=xqTb[:, :, :],
                             func=AF.Copy)
    else:
        # PE transpose (bf16) into PSUM in two half-tiles (1 bank each),
        # ACT copy-cast -> fp8
        for h in range(2):
            tp = cx.tpp.tile([128, KH, 128], BF16, tag="tp",
                             name=f"tp{i}_{h}")
            for j in range(KH):
                jj = h * KH + j
                nc.tensor.transpose(tp[:, j, :],
                                    xq[:, jj * 128:(jj + 1) * 128],
                                    cx.idn[:])
            nc.scalar.activation(out=xqT[:, h * KH:(h + 1) * KH, :],
                                 in_=tp[:, :, :], func=AF.Copy)
    cx.xqT[i] = xqT

    # ssq_int accumulates on the DoubleRow gram diagonal (PE)
    gram = cx.grp.tile([128, 128], F32, tag="gram", name=f"gram{i}")
    for jj in range(NP):
        nc.tensor.matmul(gram[:], xqT[:, 2 * jj:2 * jj + 2, :],
                         xqT[:, 2 * jj:2 * jj + 2, :],
                         start=(jj == 0), stop=(jj == NP - 1), perf_mode=DR)
    cx.gram[i] = gram


def _emit_finish(nc, cx, i):
    """dsc (Pool, reads gram PSUM) + os-chain (DVE tinies + ACT sqrt)."""
    st = cx.st
    dsc = cx.dscp.tile([128, 128], F32, tag="dsc", name=f"dsc{i}")
    ssq = st.tile([128, 1], F32, tag="ssq", name=f"ssq{i}")
    nc.gpsimd.scalar_tensor_tensor(out=dsc[:], in0=cx.gram[i][:], scalar=1.0,
                                   in1=cx.idn[:],
                                   op0=ALU.mult, op1=ALU.mult,
                                   accum_out=ssq[:])
    rms = st.tile([128, 1], F32, tag="rms", name=f"rms{i}")
    nc.scalar.activation(out=rms[:], in_=ssq[:], func=AF.Sqrt,
                         scale=1.0 / DIN)
    v = st.tile([128, 1], F32, tag="v", name=f"v{i}")
    nc.vector.tensor_scalar_mul(v[:], ssq[:], 1.0 / DIN)
    y0 = st.tile([128, 1], F32, tag="y0", name=f"y0_{i}")
    nc.vector.reciprocal(y0[:], rms[:])
    a2 = st.tile([128, 1], F32, tag="a2", name=f"a2_{i}")
    nc.vector.tensor_mul(a2[:], y0[:], y0[:])
    bq = st.tile([128, 1], F32, tag="bq", name=f"bq{i}")
    nc.vector.tensor_mul(bq[:], v[:], a2[:])
    cq = st.tile([128, 1], F32, tag="cq", name=f"cq{i}")
    nc.vector.tensor_scalar(out=cq[:], in0=bq[:], scalar1=-0.5, scalar2=1.5,
                            op0=ALU.mult, op1=ALU.add)
    osc = st.tile([128, 1], F32, tag="os", name=f"os{i}")
    nc.vector.tensor_mul(osc[:], y0[:], cq[:])
    negos = st.tile([128, 1], F32, tag="negos", name=f"negos{i}")
    nc.vector.tensor_scalar_mul(negos[:], osc[:], -1.0)
    b_col = st.tile([128, 1], F32, tag="b", name=f"b{i}")
    nc.vector.tensor_mul(b_col[:], cx.S[i][:], osc[:])
    cx.negos[i] = negos
    cx.b[i] = b_col


def _emit_drain_q(nc, cx, i, q):
    """(S - ps)*os for quarter q of tile i -> ob bf16."""
    ps = cx.ps[i][q]
    oslice = cx.ob[i][:, q * QW:(q + 1) * QW]
    eng = DRAIN_ENG[q]
    if eng == "act":
        nc.scalar.activation(out=oslice, in_=ps[:], func=AF.Identity,
                             bias=cx.b[i][:], scale=cx.negos[i][:])
    elif eng == "dve":
        nc.vector.tensor_scalar(out=oslice, in0=ps[:],
                                scalar1=cx.S[i][:], scalar2=cx.negos[i][:],
                                op0=ALU.subtract, op1=ALU.mult)
    else:
        nc.gpsimd.tensor_scalar(out=oslice, in0=ps[:],
                                scalar1=cx.S[i][:], scalar2=cx.negos[i][:],
                                op0=ALU.subtract, op1=ALU.mult)


def _emit_mm_burst(nc, cx, i):
    """All 32 DoubleRow matmuls of tile i, quarter-major, + drains."""
    xqT = cx.xqT[i]
    cx.ps[i] = {}
    for q in range(4):
        ps = cx.mmp.tile([128, QW], F32, tag="mm", name=f"ps{i}_{q}")
        for p in range(NP):
            nc.tensor.matmul(ps[:], xqT[:, 2 * p:2 * p + 2, :],
                             cx.wcT[p][:, :, q * QW:(q + 1) * QW],
                             start=(p == 0), stop=(p == NP - 1), perf_mode=DR)
        cx.ps[i][q] = ps
        _emit_drain_q(nc, cx, i, q)


def _emit_mm_pairstream(nc, cx, i):
    """Tile i matmuls pair-major: consumes W pairs as they are quantized
    during the W-load phase (holds its 4 psum banks until the last pair)."""
    xqT = cx.xqT[i]
    cx.ps[i] = {q: cx.mmp.tile([128, QW], F32, tag="mm", name=f"ps{i}_{q}")
                for q in range(4)}
    for n, p in enumerate(PAIR_ORDER):
        for q in range(4):
            nc.tensor.matmul(cx.ps[i][q][:], xqT[:, 2 * p:2 * p + 2, :],
                             cx.wcT[p][:, :, q * QW:(q + 1) * QW],
                             start=(n == 0), stop=(n == NP - 1), perf_mode=DR)
    for q in range(4):
        _emit_drain_q(nc, cx, i, q)


def build():
    nc = bacc.Bacc("TRN2", target_bir_lowering=False, debug=False,
                   num_devices=NCORES)
    cx = Ctx()
    cx.x_d = nc.dram_tensor("x", [T, DIN], F32, kind="ExternalInput")
    cx.wT_d = nc.dram_tensor("wT", [DIN, DOUT], F32, kind="ExternalInput")
    cx.out_d = nc.dram_tensor("out", [T, DOUT], BF16, kind="ExternalOutput")
    cx.xqT, cx.negos, cx.b, cx.S = {}, {}, {}, {}
    cx.ps, cx.xf, cx.ob, cx.y, cx.gram = {}, {}, {}, {}, {}

    with tile.TileContext(nc) as tc:
        with (
            tc.tile_pool(name="singles", bufs=1) as singles,
            tc.tile_pool(name="wf", bufs=10) as wfp,
            tc.tile_pool(name="xf", bufs=4) as xfp,
            tc.tile_pool(name="y", bufs=2) as yp,
            tc.tile_pool(name="xq", bufs=2) as xqp,
            tc.tile_pool(name="xqT", bufs=3) as xqTp,
            tc.tile_pool(name="xqTb", bufs=2) as xqTbp,
            tc.tile_pool(name="dsc", bufs=1) as dscp,
            tc.tile_pool(name="st", bufs=24) as st,
            tc.tile_pool(name="outp", bufs=5) as outp,
            tc.tile_pool(name="mmps", bufs=4, space="PSUM") as mmp,
            tc.tile_pool(name="tpps", bufs=2, space="PSUM") as tpp,
            tc.tile_pool(name="grps", bufs=2, space="PSUM") as grp,
        ):
            cx.yp, cx.xqp, cx.xqTp, cx.xqTbp = yp, xqp, xqTp, xqTbp
            cx.st, cx.outp, cx.dscp = st, outp, dscp
            cx.mmp, cx.tpp, cx.grp = mmp, tpp, grp

            # ---- preamble
            dummy = singles.tile([128, 1], F32)
            nc.vector.memset(dummy[:], 1.0)
            dummy2 = singles.tile([128, 1], F32)
            for fn in (AF.Sqrt, AF.Identity, AF.Copy):
                nc.scalar.activation(out=dummy2[:], in_=dummy[:], func=fn)
            cx.idn = singles.tile([128, 128], BF16)
            make_identity(nc, cx.idn[:])
            cx.c_col = singles.tile([128, 1], F32)
            nc.vector.memset(cx.c_col[:], C_MAGIC)

            # ---- SP queue: w0, w1 first (gamma chunks).  Pool queue: the
            # first three x tiles as fp16 casting DMAs.
            wf = {}
            for j in range(2):
                wf[j] = wfp.tile([128, DOUT], F32, tag="wf", name=f"w_{j}")
                nc.sync.dma_start(wf[j][:],
                                  cx.wT_d.ap()[j * 128:(j + 1) * 128, :])

            def emit_xload(i):
                cx.xf[i] = xfp.tile([128, DIN], FP16, tag="xf",
                                    name=f"xf{i}")
                nc.gpsimd.dma_start(cx.xf[i][:],
                                    cx.x_d.ap()[i * 128:(i + 1) * 128, :])

            for i in range(3):
                emit_xload(i)

            # ---- gamma accumulation passes on ACT
            wsum = singles.tile([128, 2], F32)
            sc0 = yp.tile([128, DOUT], F32, tag="y", name="wabs_s0")
            nc.scalar.activation(out=sc0[:], in_=wf[0][:], func=AF.Identity,
                                 accum_out=wsum[:, 0:1])
            sc1 = yp.tile([128, DOUT], F32, tag="y", name="wabs_s1")
            nc.scalar.activation(out=sc1[:], in_=wf[1][:], func=AF.Identity,
                                 accum_out=wsum[:, 1:2])

            # ---- tile 0 chain (front half)
            _emit_chain_front(nc, cx, 0)

            # ---- collective: AllGather of 8 local-slice sums
            wsum1 = singles.tile([128, 1], F32)
            nc.vector.tensor_reduce(out=wsum1[:], in_=wsum[:],
                                    axis=mybir.AxisListType.X, op=ALU.add)
            total_loc = singles.tile([128, 1], F32)
            nc.gpsimd.partition_all_reduce(total_loc[:], wsum1[:],
                                           channels=128,
                                           reduce_op=bass_isa.ReduceOp.add)
            cc_in = singles.tile([1, 1], F32, space="DRAM")
            cc_out = singles.tile([1, NCORES], F32, space="DRAM")
            cx.ccst_inst = nc.gpsimd.dma_start(cc_in[:], total_loc[0:1, 0:1])
            nc.gpsimd.collective_compute(
                "AllGather", ALU.bypass,
                replica_groups=[list(range(NCORES))],
                ins=[cc_in[:]], outs=[cc_out[:]])

            # tiles 1,2 chains fill the 15us collective window (their y ops
            # sit between the CC dispatch and the result load in the Pool
            # queue)
            _emit_chain_front(nc, cx, 1)
            _emit_chain_front(nc, cx, 2)
            emit_xload(3)

            gsum = singles.tile([1, NCORES], F32)
            cx.ccld_inst = nc.gpsimd.dma_start(gsum[:], cc_out[:])
            gtot = singles.tile([1, 1], F32)
            nc.vector.tensor_reduce(out=gtot[:], in_=gsum[:],
                                    axis=mybir.AxisListType.X, op=ALU.add)
            thr0 = singles.tile([1, 1], F32)
            nc.gpsimd.tensor_scalar(out=thr0[:], in0=gtot[:],
                                    scalar1=0.5 / (DIN * DOUT),
                                    scalar2=0.5 * EPS_GAMMA,
                                    op0=ALU.mult, op1=ALU.add)
            thr = singles.tile([128, 1], F32)
            nc.gpsimd.partition_broadcast(thr[:], thr0[:])

            # ---- W loads w2..w15 (SP queue).  Bulk loads yield the DMA
            # pool to the tiny collective bounce transfers via dep edges.
            from concourse.tile_rust import add_dep_helper
            for j in range(2, KC):
                wf[j] = wfp.tile([128, DOUT], F32, tag="wf", name=f"w_{j}")
                ld = nc.sync.dma_start(wf[j][:],
                                       cx.wT_d.ap()[j * 128:(j + 1) * 128, :])
                if j >= 4:
                    add_dep_helper(ld.ins, cx.ccst_inst.ins, sync=True,
                                   reason="yield DMA pool to cc_in store")
                if j >= 10:
                    add_dep_helper(ld.ins, cx.ccld_inst.ins, sync=True,
                                   reason="yield DMA pool to cc_out load")

            # ---- W quantization: wc = (w <= thr) -> fp8
            cx.wcT = {p: singles.tile([128, 2, DOUT], FP8, name=f"wcT{p}")
                      for p in range(NP)}

            def emit_quant(j):
                eng = {"dve": nc.vector, "pool": nc.gpsimd}[QUANT_ENG[j]]
                eng.tensor_scalar(out=cx.wcT[j // 2][:, j % 2, :],
                                  in0=wf[j][:], scalar1=thr[:], scalar2=None,
                                  op0=ALU.is_le)

            for j in range(KC):
                if QUANT_ENG[j] == "dve":
                    emit_quant(j)
            # finishes for tiles 0-2 (Pool dsc sits after bcast, before the
            # Pool quant block; their os values are needed by drains ~52us)
            for i in range(3):
                _emit_finish(nc, cx, i)
            for j in range(KC):
                if QUANT_ENG[j] == "pool":
                    emit_quant(j)
            emit_xload(4)

            # ---- tile 0 matmuls pair-streamed during the W phase
            cx.ob[0] = outp.tile([128, DOUT], BF16, tag="ob", name="ob0")
            _emit_mm_pairstream(nc, cx, 0)
            for i in (1, 2):
                cx.ob[i] = outp.tile([128, DOUT], BF16, tag="ob",
                                     name=f"ob{i}")
                _emit_mm_burst(nc, cx, i)

            # ---- steady state: iteration i emits finish(i-1), burst(i-1),
            # then chain(i); x loads lead by 2; stores trail by 3
            stores_emitted = 0

            def emit_store(i):
                nc.sync.dma_start(cx.out_d.ap()[i * 128:(i + 1) * 128, :],
                                  cx.ob[i][:])

            for i in range(3, TP):
                if i + 2 < TP:
                    emit_xload(i + 2)
                _emit_finish(nc, cx, i - 1)
                _emit_mm_burst(nc, cx, i - 1)
                while stores_emitted < i - 3:
                    emit_store(stores_emitted)
                    stores_emitted += 1
                _emit_chain_front(nc, cx, i)
                cx.ob[i] = outp.tile([128, DOUT], BF16, tag="ob",
                                     name=f"ob{i}")
            _emit_finish(nc, cx, TP - 1)
            _emit_mm_burst(nc, cx, TP - 1)
            while stores_emitted < TP:
                emit_store(stores_emitted)
                stores_emitted += 1

    nc.compile()
    return nc


_NC_CACHE = []


def kernel(x: np.ndarray, weight: np.ndarray) -> np.ndarray:
    assert x.shape == (B, S, DIN) and weight.shape == (DOUT, DIN)
    if not _NC_CACHE:
        _NC_CACHE.append(build())
    nc = _NC_CACHE[0]

    xs = np.ascontiguousarray(x.reshape(B * S, DIN), dtype=np.float32)
    wT = np.ascontiguousarray(weight.T.astype(np.float32))
    in_maps = []
    for k in range(NCORES):
        r = 256 * k
        # roll the contraction dim on both operands so chunks 0,1 of this
        # core's wT stream are its private gamma slice (no separate wg load)
        wk = np.ascontiguousarray(np.roll(wT, -r, axis=0))
        xk = np.ascontiguousarray(np.roll(xs[k * T:(k + 1) * T], -r, axis=1))
        in_maps.append({"x": xk, "wT": wk})
    res = run_bass_kernel_spmd(nc, in_maps, core_ids=list(range(NCORES)))
    out = np.concatenate(
        [np.asarray(res.results[k]["out"]).astype(np.float32)
         for k in range(NCORES)], axis=0)
    return np.ascontiguousarray(out.reshape(B, S, DOUT))


# revision 26
# speedup vs baseline: 1.1893x; 1.1893x over previous
"""BitLinear forward kernel for Trainium2 (8 NeuronCores, data-parallel),
fp8 DoubleRow edition, v3 schedule.

Forward math of the reference (straight-through estimators resolved):
    out = (x_quant/scale) @ w_q^T
    x_int = round(x_norm * 127/amax_norm) = round(x * 127/amax)   (rms cancels)
    x_quant/scale = x_int * amax/(127*rms)
    w_q = (w > 0.5*(gamma+eps)) in {0,1}     (w >= 0 here)

Device scheme (per core, 2048 tokens, 16 token tiles of 128):
  * complement weights Wc = 1 - w_q = (w <= thr): density ~0.25, so the fp8
    quantization error of x only flows through 1/4 of the terms:
        out = (S - x_int @ Wc) * os
  * x_int via the magic-constant RNE trick; S[t] = sum_d x_int fused into the
    quantize pass (accum_out).  x_int cast fp8 via PE transpose + ACT copy;
    products with {0,1} and f32 PSUM accumulation keep the matmul exact given
    the fp8 rounding of x.  Matmuls fp8 MatmulPerfMode.DoubleRow (0.5cyc/row).
  * per-token scale os = 1/sqrt(ssq_int/2048) from the DoubleRow gram diag.
  * gamma = mean|W| distributed: host rolls each core's wT (and x's feature
    dim identically) so chunks 0,1 of the core's stream are its private
    256-row slice; local slice sum -> tiny AllGather (15us vs 28us for
    AllReduce) -> total.  No separate wg input.

v3 schedule:
  * W owns the DMA early: single SP queue ordered w0,w1,x0,w2..w15,x1..x15
    with the first out stores interleaved between the last x loads.  The
    collective bounce DMAs jump the bulk-load FIFO via dep edges.  All 8
    quantized weight pairs exist by ~55us, so the PSUM accumulation never
    holds banks waiting for a late pair.
  * Weight quant split DVE (chunks 0-5, 12-15; 2x tensor_scalar mode) and
    Pool (6-11) during the x-starved W phase.
  * Crunch loadout per tile: DVE amax+xq+dsc+1 drain, ACT cast+3 drains,
    Pool y, PE transposes+gram+32 DR matmuls.  Tile 0's matmuls are
    pair-streamed into the otherwise idle PE during the W phase.
"""
import numpy as np

import concourse.bass as bass
import concourse.bacc as bacc
import concourse.bass_isa as bass_isa
import concourse.mybir as mybir
import concourse.tile as tile
from concourse.bass_utils import run_bass_kernel_spmd
from concourse.masks import make_identity

F32 = mybir.dt.float32
BF16 = mybir.dt.bfloat16
FP8 = mybir.dt.float8e4
DR = mybir.MatmulPerfMode.DoubleRow
AF = mybir.ActivationFunctionType
ALU = mybir.AluOpType

NCORES = 8
B, S, DIN, DOUT = 4, 4096, 2048, 2048
T = (B * S) // NCORES        # tokens per core = 2048
TP = T // 128                # token tiles per core = 16
KC = DIN // 128              # contraction chunks = 16
NP = KC // 2                 # DoubleRow k-pairs = 8
QW = DOUT // 4               # psum quarter width = 512

C_MAGIC = 12582912.0         # 1.5 * 2**23: fp32 round-to-nearest-even trick
EPS_GAMMA = 1e-5

# which engine quantizes W chunk j
QUANT_ENG = {0: "dve", 1: "dve", 2: "dve", 3: "dve", 4: "dve", 5: "dve",
             6: "pool", 7: "pool", 8: "pool", 9: "pool", 10: "pool",
             11: "pool", 12: "dve", 13: "dve", 14: "dve", 15: "dve"}
# pair consumption order for the pair-streamed tile 0 (by readiness)
PAIR_ORDER = [0, 1, 3, 2, 4, 5, 6, 7]
# drain engine per psum quarter
DRAIN_ENG = ["act", "act", "dve", "act"]
# after this many trailing x loads, interleave 2 stores per load
ST_INTERLEAVE_AT = 12


class Ctx:
    pass


def _emit_x_chain(nc, cx, i):
    """Token tile i compute chain up to (not incl.) the output matmuls."""
    st = cx.st
    xf = cx.xf[i]

    amax = st.tile([128, 1], F32, tag="amax", name=f"amax{i}")
    nc.vector.tensor_reduce(out=amax[:], in_=xf[:], axis=mybir.AxisListType.X,
                            op=ALU.max, apply_absolute_value=True)
    # m = 127/amax, Newton-refined (tiny [128,1] ops are ~free)
    rcp = st.tile([128, 1], F32, tag="rcp", name=f"rcp{i}")
    nc.vector.reciprocal(rcp[:], amax[:])
    t0 = st.tile([128, 1], F32, tag="t0", name=f"t0_{i}")
    nc.vector.tensor_mul(t0[:], amax[:], rcp[:])
    u0 = st.tile([128, 1], F32, tag="u0", name=f"u0_{i}")
    nc.vector.tensor_scalar(out=u0[:], in0=t0[:], scalar1=2.0, scalar2=-1.0,
                            op0=ALU.subtract, op1=ALU.mult)
    rcp1 = st.tile([128, 1], F32, tag="rcp1", name=f"rcp1_{i}")
    nc.vector.tensor_mul(rcp1[:], rcp[:], u0[:])
    m = st.tile([128, 1], F32, tag="m", name=f"m{i}")
    nc.vector.tensor_scalar_mul(m[:], rcp1[:], 127.0)

    # y = x*m + C on Pool (its only per-tile duty)
    y = cx.yp.tile([128, DIN], F32, tag="y", name=f"y{i}")
    nc.gpsimd.tensor_scalar(out=y[:], in0=xf[:], scalar1=m[:],
                            scalar2=C_MAGIC, op0=ALU.mult, op1=ALU.add)
    # xq = y - C -> bf16 ints, S = sum_d xq (DVE 2x + free accum)
    xq = cx.xqp.tile([128, DIN], BF16, tag="xq", name=f"xq{i}")
    S_col = st.tile([128, 1], F32, tag="S", name=f"S{i}")
    nc.vector.tensor_scalar(out=xq[:], in0=y[:],
                            scalar1=C_MAGIC, scalar2=0.0,
                            op0=ALU.subtract, op1=ALU.add,
                            accum_out=S_col[:])

    # PE transpose (bf16) into PSUM, ACT copy-cast -> fp8 SBUF
    tp = cx.tpp.tile([128, KC, 128], BF16, tag="tp", name=f"tp{i}")
    for j in range(KC):
        nc.tensor.transpose(tp[:, j, :], xq[:, j * 128:(j + 1) * 128],
                            cx.idn[:])
    xqT = cx.xqTp.tile([128, KC, 128], FP8, tag="xqT", name=f"xqT{i}")
    nc.scalar.activation(out=xqT[:, :, :], in_=tp[:, :, :], func=AF.Copy)
    cx.xqT[i] = xqT

    # ssq_int from the DoubleRow gram diagonal
    gram = cx.grp.tile([128, 128], F32, tag="gram", name=f"gram{i}")
    for jj in range(NP):
        nc.tensor.matmul(gram[:], xqT[:, 2 * jj:2 * jj + 2, :],
                         xqT[:, 2 * jj:2 * jj + 2, :],
                         start=(jj == 0), stop=(jj == NP - 1), perf_mode=DR)
    dsc = cx.dscp.tile([128, 128], F32, tag="dsc", name=f"dsc{i}")
    ssq = st.tile([128, 1], F32, tag="ssq", name=f"ssq{i}")
    nc.vector.scalar_tensor_tensor(out=dsc[:], in0=gram[:], scalar=1.0,
                                   in1=cx.idn[:],
                                   op0=ALU.mult, op1=ALU.mult,
                                   accum_out=ssq[:])
    # os = 1/sqrt(ssq/DIN): ACT sqrt + DVE recip + one rsqrt-Newton step
    rms = st.tile([128, 1], F32, tag="rms", name=f"rms{i}")
    nc.scalar.activation(out=rms[:], in_=ssq[:], func=AF.Sqrt,
                         scale=1.0 / DIN)
    v = st.tile([128, 1], F32, tag="v", name=f"v{i}")
    nc.vector.tensor_scalar_mul(v[:], ssq[:], 1.0 / DIN)
    y0 = st.tile([128, 1], F32, tag="y0", name=f"y0_{i}")
    nc.vector.reciprocal(y0[:], rms[:])
    a2 = st.tile([128, 1], F32, tag="a2", name=f"a2_{i}")
    nc.vector.tensor_mul(a2[:], y0[:], y0[:])
    bq = st.tile([128, 1], F32, tag="bq", name=f"bq{i}")
    nc.vector.tensor_mul(bq[:], v[:], a2[:])
    cq = st.tile([128, 1], F32, tag="cq", name=f"cq{i}")
    nc.vector.tensor_scalar(out=cq[:], in0=bq[:], scalar1=-0.5, scalar2=1.5,
                            op0=ALU.mult, op1=ALU.add)
    osc = st.tile([128, 1], F32, tag="os", name=f"os{i}")
    nc.vector.tensor_mul(osc[:], y0[:], cq[:])
    negos = st.tile([128, 1], F32, tag="negos", name=f"negos{i}")
    nc.vector.tensor_scalar_mul(negos[:], osc[:], -1.0)
    b_col = st.tile([128, 1], F32, tag="b", name=f"b{i}")
    nc.vector.tensor_mul(b_col[:], S_col[:], osc[:])
    cx.negos[i] = negos
    cx.b[i] = b_col
    cx.S[i] = S_col


def _emit_drain_q(nc, cx, i, q):
    """(S - ps)*os for quarter q of tile i -> ob bf16."""
    ps = cx.ps[i][q]
    oslice = cx.ob[i][:, q * QW:(q + 1) * QW]
    if DRAIN_ENG[q] == "act":
        nc.scalar.activation(out=oslice, in_=ps[:], func=AF.Identity,
                             bias=cx.b[i][:], scale=cx.negos[i][:])
    else:
        nc.vector.tensor_scalar(out=oslice, in0=ps[:],
                                scalar1=cx.S[i][:], scalar2=cx.negos[i][:],
                                op0=ALU.subtract, op1=ALU.mult)


def _emit_mm_burst(nc, cx, i):
    """All 32 DoubleRow matmuls of tile i, quarter-major, + drains."""
    xqT = cx.xqT[i]
    cx.ps[i] = {}
    for q in range(4):
        ps = cx.mmp.tile([128, QW], F32, tag="mm", name=f"ps{i}_{q}")
        for p in range(NP):
            nc.tensor.matmul(ps[:], xqT[:, 2 * p:2 * p + 2, :],
                             cx.wcT[p][:, :, q * QW:(q + 1) * QW],
                             start=(p == 0), stop=(p == NP - 1), perf_mode=DR)
        cx.ps[i][q] = ps
        _emit_drain_q(nc, cx, i, q)


def _emit_mm_pairstream(nc, cx, i):
    """Tile i matmuls pair-major: consumes W pairs as they are quantized
    during the W-load phase (holds its 4 psum banks until the last pair)."""
    xqT = cx.xqT[i]
    cx.ps[i] = {q: cx.mmp.tile([128, QW], F32, tag="mm", name=f"ps{i}_{q}")
                for q in range(4)}
    for n, p in enumerate(PAIR_ORDER):
        for q in range(4):
            nc.tensor.matmul(cx.ps[i][q][:], xqT[:, 2 * p:2 * p + 2, :],
                             cx.wcT[p][:, :, q * QW:(q + 1) * QW],
                             start=(n == 0), stop=(n == NP - 1), perf_mode=DR)
    for q in range(4):
        _emit_drain_q(nc, cx, i, q)


def build():
    nc = bacc.Bacc("TRN2", target_bir_lowering=False, debug=False,
                   num_devices=NCORES)
    cx = Ctx()
    cx.x_d = nc.dram_tensor("x", [T, DIN], F32, kind="ExternalInput")
    cx.wT_d = nc.dram_tensor("wT", [DIN, DOUT], F32, kind="ExternalInput")
    cx.out_d = nc.dram_tensor("out", [T, DOUT], BF16, kind="ExternalOutput")
    cx.xqT, cx.negos, cx.b, cx.S = {}, {}, {}, {}
    cx.ps, cx.xf, cx.ob = {}, {}, {}

    with tile.TileContext(nc) as tc:
        with (
            tc.tile_pool(name="singles", bufs=1) as singles,
            tc.tile_pool(name="wf", bufs=10) as wfp,
            tc.tile_pool(name="x", bufs=3) as xp,
            tc.tile_pool(name="y", bufs=2) as yp,
            tc.tile_pool(name="xq", bufs=1) as xqp,
            tc.tile_pool(name="xqT", bufs=3) as xqTp,
            tc.tile_pool(name="dsc", bufs=1) as dscp,
            tc.tile_pool(name="st", bufs=24) as st,
            tc.tile_pool(name="outp", bufs=6) as outp,
            tc.tile_pool(name="mmps", bufs=4, space="PSUM") as mmp,
            tc.tile_pool(name="tpps", bufs=1, space="PSUM") as tpp,
            tc.tile_pool(name="grps", bufs=2, space="PSUM") as grp,
        ):
            cx.xp, cx.yp, cx.xqp, cx.xqTp = xp, yp, xqp, xqTp
            cx.st, cx.outp, cx.dscp = st, outp, dscp
            cx.mmp, cx.tpp, cx.grp = mmp, tpp, grp

            # ---- preamble
            dummy = singles.tile([128, 1], F32)
            nc.vector.memset(dummy[:], 1.0)
            dummy2 = singles.tile([128, 1], F32)
            for fn in (AF.Sqrt, AF.Identity, AF.Copy):
                nc.scalar.activation(out=dummy2[:], in_=dummy[:], func=fn)
            cx.idn = singles.tile([128, 128], BF16)
            make_identity(nc, cx.idn[:])
            cx.c_col = singles.tile([128, 1], F32)
            nc.vector.memset(cx.c_col[:], C_MAGIC)

            # ---- SP DMA queue order: w0, w1 (gamma chunks first), x0
            wf = {}
            for j in range(2):
                wf[j] = wfp.tile([128, DOUT], F32, tag="wf", name=f"w_{j}")
                nc.sync.dma_start(wf[j][:],
                                  cx.wT_d.ap()[j * 128:(j + 1) * 128, :])
            cx.xf[0] = xp.tile([128, DIN], F32, tag="xf", name="xf0")
            nc.sync.dma_start(cx.xf[0][:], cx.x_d.ap()[0:128, :])

            # ---- gamma accumulation passes on ACT
            wsum = singles.tile([128, 2], F32)
            sc0 = yp.tile([128, DOUT], F32, tag="y", name="wabs_s0")
            nc.scalar.activation(out=sc0[:], in_=wf[0][:], func=AF.Identity,
                                 accum_out=wsum[:, 0:1])
            sc1 = yp.tile([128, DOUT], F32, tag="y", name="wabs_s1")
            nc.scalar.activation(out=sc1[:], in_=wf[1][:], func=AF.Identity,
                                 accum_out=wsum[:, 1:2])

            # ---- tile 0 chain (heads every engine queue)
            _emit_x_chain(nc, cx, 0)

            # ---- collective: AllGather of 8 local-slice sums
            wsum1 = singles.tile([128, 1], F32)
            nc.vector.tensor_reduce(out=wsum1[:], in_=wsum[:],
                                    axis=mybir.AxisListType.X, op=ALU.add)
            total_loc = singles.tile([128, 1], F32)
            nc.gpsimd.partition_all_reduce(total_loc[:], wsum1[:],
                                           channels=128,
                                           reduce_op=bass_isa.ReduceOp.add)
            cc_in = singles.tile([1, 1], F32, space="DRAM")
            cc_out = singles.tile([1, NCORES], F32, space="DRAM")
            cx.ccst_inst = nc.gpsimd.dma_start(cc_in[:], total_loc[0:1, 0:1])
            nc.gpsimd.collective_compute(
                "AllGather", ALU.bypass,
                replica_groups=[list(range(NCORES))],
                ins=[cc_in[:]], outs=[cc_out[:]])
            gsum = singles.tile([1, NCORES], F32)
            cx.ccld_inst = nc.gpsimd.dma_start(gsum[:], cc_out[:])
            gtot = singles.tile([1, 1], F32)
            nc.vector.tensor_reduce(out=gtot[:], in_=gsum[:],
                                    axis=mybir.AxisListType.X, op=ALU.add)
            thr0 = singles.tile([1, 1], F32)
            nc.gpsimd.tensor_scalar(out=thr0[:], in0=gtot[:],
                                    scalar1=0.5 / (DIN * DOUT),
                                    scalar2=0.5 * EPS_GAMMA,
                                    op0=ALU.mult, op1=ALU.add)
            thr = singles.tile([128, 1], F32)
            nc.gpsimd.partition_broadcast(thr[:], thr0[:])

            # ---- W loads w2..w15 (SP queue).  Bulk loads yield the DMA
            # pool to the tiny collective bounce transfers via dep edges.
            from concourse.tile_rust import add_dep_helper
            for j in range(2, KC):
                wf[j] = wfp.tile([128, DOUT], F32, tag="wf", name=f"w_{j}")
                ld = nc.sync.dma_start(wf[j][:],
                                       cx.wT_d.ap()[j * 128:(j + 1) * 128, :])
                if j >= 4:
                    add_dep_helper(ld.ins, cx.ccst_inst.ins, sync=True,
                                   reason="yield DMA pool to cc_in store")
                if j >= 9:
                    add_dep_helper(ld.ins, cx.ccld_inst.ins, sync=True,
                                   reason="yield DMA pool to cc_out load")

            # ---- W quantization: wc = (w <= thr) -> fp8
            cx.wcT = {p: singles.tile([128, 2, DOUT], FP8, name=f"wcT{p}")
                      for p in range(NP)}

            def emit_quant(j):
                eng = {"dve": nc.vector, "pool": nc.gpsimd}[QUANT_ENG[j]]
                eng.tensor_scalar(out=cx.wcT[j // 2][:, j % 2, :],
                                  in0=wf[j][:], scalar1=thr[:], scalar2=None,
                                  op0=ALU.is_le)

            for j in range(KC):
                if QUANT_ENG[j] == "dve":
                    emit_quant(j)
            for j in range(KC):
                if QUANT_ENG[j] == "pool":
                    emit_quant(j)

            # ---- tile 0 matmuls pair-streamed during the W phase
            cx.ob[0] = outp.tile([128, DOUT], BF16, tag="ob", name="ob0")
            _emit_mm_pairstream(nc, cx, 0)

            # ---- x1..x15 + crunch
            stores_emitted = 0

            def emit_store(i):
                nc.sync.dma_start(cx.out_d.ap()[i * 128:(i + 1) * 128, :],
                                  cx.ob[i][:])

            for i in range(1, TP):
                cx.xf[i] = xp.tile([128, DIN], F32, tag="xf", name=f"xf{i}")
                nc.sync.dma_start(cx.xf[i][:],
                                  cx.x_d.ap()[i * 128:(i + 1) * 128, :])
                if i >= ST_INTERLEAVE_AT:
                    for _ in range(2):
                        if stores_emitted < i - 3:
                            emit_store(stores_emitted)
                            stores_emitted += 1
                _emit_x_chain(nc, cx, i)
                cx.ob[i] = outp.tile([128, DOUT], BF16, tag="ob",
                                     name=f"ob{i}")
                if i >= 2:
                    _emit_mm_burst(nc, cx, i - 1)
            _emit_mm_burst(nc, cx, TP - 1)
            while stores_emitted < TP:
                emit_store(stores_emitted)
                stores_emitted += 1

    nc.compile()
    return nc


_NC_CACHE = []


def kernel(x: np.ndarray, weight: np.ndarray) -> np.ndarray:
    assert x.shape == (B, S, DIN) and weight.shape == (DOUT, DIN)
    if not _NC_CACHE:
        _NC_CACHE.append(build())
    nc = _NC_CACHE[0]

    xs = np.ascontiguousarray(x.reshape(B * S, DIN), dtype=np.float32)
    wT = np.ascontiguousarray(weight.T.astype(np.float32))
    in_maps = []
    for k in range(NCORES):
        r = 256 * k
        # roll the contraction dim on both operands so chunks 0,1 of this
        # core's wT stream are its private gamma slice (no separate wg load)
        wk = np.ascontiguousarray(np.roll(wT, -r, axis=0))
        xk = np.ascontiguousarray(np.roll(xs[k * T:(k + 1) * T], -r, axis=1))
        in_maps.append({"x": xk, "wT": wk})
    res = run_bass_kernel_spmd(nc, in_maps, core_ids=list(range(NCORES)))
    out = np.concatenate(
        [np.asarray(res.results[k]["out"]).astype(np.float32)
         for k in range(NCORES)], axis=0)
    return np.ascontiguousarray(out.reshape(B, S, DOUT))


# revision 27
# speedup vs baseline: 1.1903x; 1.0009x over previous
"""BitLinear forward kernel for Trainium2 (8 NeuronCores, data-parallel),
fp8 DoubleRow edition, v3 schedule.

Forward math of the reference (straight-through estimators resolved):
    out = (x_quant/scale) @ w_q^T
    x_int = round(x_norm * 127/amax_norm) = round(x * 127/amax)   (rms cancels)
    x_quant/scale = x_int * amax/(127*rms)
    w_q = (w > 0.5*(gamma+eps)) in {0,1}     (w >= 0 here)

Device scheme (per core, 2048 tokens, 16 token tiles of 128):
  * complement weights Wc = 1 - w_q = (w <= thr): density ~0.25, so the fp8
    quantization error of x only flows through 1/4 of the terms:
        out = (S - x_int @ Wc) * os
  * x_int via the magic-constant RNE trick; S[t] = sum_d x_int fused into the
    quantize pass (accum_out).  x_int cast fp8 via PE transpose + ACT copy;
    products with {0,1} and f32 PSUM accumulation keep the matmul exact given
    the fp8 rounding of x.  Matmuls fp8 MatmulPerfMode.DoubleRow (0.5cyc/row).
  * per-token scale os = 1/sqrt(ssq_int/2048) from the DoubleRow gram diag.
  * gamma = mean|W| distributed: host rolls each core's wT (and x's feature
    dim identically) so chunks 0,1 of the core's stream are its private
    256-row slice; local slice sum -> tiny AllGather (15us vs 28us for
    AllReduce) -> total.  No separate wg input.

v3 schedule:
  * W owns the DMA early: single SP queue ordered w0,w1,x0,w2..w15,x1..x15
    with the first out stores interleaved between the last x loads.  The
    collective bounce DMAs jump the bulk-load FIFO via dep edges.  All 8
    quantized weight pairs exist by ~55us, so the PSUM accumulation never
    holds banks waiting for a late pair.
  * Weight quant split DVE (chunks 0-5, 12-15; 2x tensor_scalar mode) and
    Pool (6-11) during the x-starved W phase.
  * Crunch loadout per tile: DVE amax+xq+dsc+1 drain, ACT cast+3 drains,
    Pool y, PE transposes+gram+32 DR matmuls.  Tile 0's matmuls are
    pair-streamed into the otherwise idle PE during the W phase.
"""
import numpy as np

import concourse.bass as bass
import concourse.bacc as bacc
import concourse.bass_isa as bass_isa
import concourse.mybir as mybir
import concourse.tile as tile
from concourse.bass_utils import run_bass_kernel_spmd
from concourse.masks import make_identity

F32 = mybir.dt.float32
BF16 = mybir.dt.bfloat16
FP8 = mybir.dt.float8e4
DR = mybir.MatmulPerfMode.DoubleRow
AF = mybir.ActivationFunctionType
ALU = mybir.AluOpType

NCORES = 8
B, S, DIN, DOUT = 4, 4096, 2048, 2048
T = (B * S) // NCORES        # tokens per core = 2048
TP = T // 128                # token tiles per core = 16
KC = DIN // 128              # contraction chunks = 16
NP = KC // 2                 # DoubleRow k-pairs = 8
QW = DOUT // 4               # psum quarter width = 512

C_MAGIC = 12582912.0         # 1.5 * 2**23: fp32 round-to-nearest-even trick
EPS_GAMMA = 1e-5

# which engine quantizes W chunk j
QUANT_ENG = {0: "dve", 1: "dve", 2: "dve", 3: "dve", 4: "dve", 5: "dve",
             6: "pool", 7: "pool", 8: "pool", 9: "pool", 10: "pool",
             11: "pool", 12: "dve", 13: "dve", 14: "dve", 15: "dve"}
# pair consumption order for the pair-streamed tile 0 (by readiness)
PAIR_ORDER = [0, 1, 3, 2, 4, 5, 6, 7]
# drain engine per psum quarter
DRAIN_ENG = ["act", "act", "dve", "act"]
# after this many trailing x loads, interleave 2 stores per load
ST_INTERLEAVE_AT = 12


class Ctx:
    pass


def _emit_x_chain(nc, cx, i):
    """Token tile i compute chain up to (not incl.) the output matmuls."""
    st = cx.st
    xf = cx.xf[i]

    amax = st.tile([128, 1], F32, tag="amax", name=f"amax{i}")
    nc.vector.tensor_reduce(out=amax[:], in_=xf[:], axis=mybir.AxisListType.X,
                            op=ALU.max, apply_absolute_value=True)
    # m = 127/amax, Newton-refined (tiny [128,1] ops are ~free)
    rcp = st.tile([128, 1], F32, tag="rcp", name=f"rcp{i}")
    nc.vector.reciprocal(rcp[:], amax[:])
    t0 = st.tile([128, 1], F32, tag="t0", name=f"t0_{i}")
    nc.vector.tensor_mul(t0[:], amax[:], rcp[:])
    u0 = st.tile([128, 1], F32, tag="u0", name=f"u0_{i}")
    nc.vector.tensor_scalar(out=u0[:], in0=t0[:], scalar1=2.0, scalar2=-1.0,
                            op0=ALU.subtract, op1=ALU.mult)
    rcp1 = st.tile([128, 1], F32, tag="rcp1", name=f"rcp1_{i}")
    nc.vector.tensor_mul(rcp1[:], rcp[:], u0[:])
    m = st.tile([128, 1], F32, tag="m", name=f"m{i}")
    nc.vector.tensor_scalar_mul(m[:], rcp1[:], 127.0)

    # y = x*m + C on Pool (its only per-tile duty)
    y = cx.yp.tile([128, DIN], F32, tag="y", name=f"y{i}")
    nc.gpsimd.tensor_scalar(out=y[:], in0=xf[:], scalar1=m[:],
                            scalar2=C_MAGIC, op0=ALU.mult, op1=ALU.add)
    # xq = y - C -> bf16 ints, S = sum_d xq (DVE 2x + free accum)
    xq = cx.xqp.tile([128, DIN], BF16, tag="xq", name=f"xq{i}")
    S_col = st.tile([128, 1], F32, tag="S", name=f"S{i}")
    nc.vector.tensor_scalar(out=xq[:], in0=y[:],
                            scalar1=C_MAGIC, scalar2=0.0,
                            op0=ALU.subtract, op1=ALU.add,
                            accum_out=S_col[:])

    # PE transpose (bf16) into PSUM, ACT copy-cast -> fp8 SBUF
    tp = cx.tpp.tile([128, KC, 128], BF16, tag="tp", name=f"tp{i}")
    for j in range(KC):
        nc.tensor.transpose(tp[:, j, :], xq[:, j * 128:(j + 1) * 128],
                            cx.idn[:])
    xqT = cx.xqTp.tile([128, KC, 128], FP8, tag="xqT", name=f"xqT{i}")
    nc.scalar.activation(out=xqT[:, :, :], in_=tp[:, :, :], func=AF.Copy)
    cx.xqT[i] = xqT

    # ssq_int from the DoubleRow gram diagonal
    gram = cx.grp.tile([128, 128], F32, tag="gram", name=f"gram{i}")
    for jj in range(NP):
        nc.tensor.matmul(gram[:], xqT[:, 2 * jj:2 * jj + 2, :],
                         xqT[:, 2 * jj:2 * jj + 2, :],
                         start=(jj == 0), stop=(jj == NP - 1), perf_mode=DR)
    dsc = cx.dscp.tile([128, 128], F32, tag="dsc", name=f"dsc{i}")
    ssq = st.tile([128, 1], F32, tag="ssq", name=f"ssq{i}")
    nc.vector.scalar_tensor_tensor(out=dsc[:], in0=gram[:], scalar=1.0,
                                   in1=cx.idn[:],
                                   op0=ALU.mult, op1=ALU.mult,
                                   accum_out=ssq[:])
    # os = 1/sqrt(ssq/DIN): ACT sqrt + DVE recip + one rsqrt-Newton step
    rms = st.tile([128, 1], F32, tag="rms", name=f"rms{i}")
    nc.scalar.activation(out=rms[:], in_=ssq[:], func=AF.Sqrt,
                         scale=1.0 / DIN)
    v = st.tile([128, 1], F32, tag="v", name=f"v{i}")
    nc.vector.tensor_scalar_mul(v[:], ssq[:], 1.0 / DIN)
    y0 = st.tile([128, 1], F32, tag="y0", name=f"y0_{i}")
    nc.vector.reciprocal(y0[:], rms[:])
    a2 = st.tile([128, 1], F32, tag="a2", name=f"a2_{i}")
    nc.vector.tensor_mul(a2[:], y0[:], y0[:])
    bq = st.tile([128, 1], F32, tag="bq", name=f"bq{i}")
    nc.vector.tensor_mul(bq[:], v[:], a2[:])
    cq = st.tile([128, 1], F32, tag="cq", name=f"cq{i}")
    nc.vector.tensor_scalar(out=cq[:], in0=bq[:], scalar1=-0.5, scalar2=1.5,
                            op0=ALU.mult, op1=ALU.add)
    osc = st.tile([128, 1], F32, tag="os", name=f"os{i}")
    nc.vector.tensor_mul(osc[:], y0[:], cq[:])
    negos = st.tile([128, 1], F32, tag="negos", name=f"negos{i}")
    nc.vector.tensor_scalar_mul(negos[:], osc[:], -1.0)
    b_col = st.tile([128, 1], F32, tag="b", name=f"b{i}")
    nc.vector.tensor_mul(b_col[:], S_col[:], osc[:])
    cx.negos[i] = negos
    cx.b[i] = b_col
    cx.S[i] = S_col


def _emit_drain_q(nc, cx, i, q):
    """(S - ps)*os for quarter q of tile i -> ob bf16."""
    ps = cx.ps[i][q]
    oslice = cx.ob[i][:, q * QW:(q + 1) * QW]
    if DRAIN_ENG[q] == "act":
        nc.scalar.activation(out=oslice, in_=ps[:], func=AF.Identity,
                             bias=cx.b[i][:], scale=cx.negos[i][:])
    else:
        nc.vector.tensor_scalar(out=oslice, in0=ps[:],
                                scalar1=cx.S[i][:], scalar2=cx.negos[i][:],
                                op0=ALU.subtract, op1=ALU.mult)


def _emit_mm_burst(nc, cx, i):
    """All 32 DoubleRow matmuls of tile i, quarter-major, + drains."""
    xqT = cx.xqT[i]
    cx.ps[i] = {}
    for q in range(4):
        ps = cx.mmp.tile([128, QW], F32, tag="mm", name=f"ps{i}_{q}")
        for p in range(NP):
            nc.tensor.matmul(ps[:], xqT[:, 2 * p:2 * p + 2, :],
                             cx.wcT[p][:, :, q * QW:(q + 1) * QW],
                             start=(p == 0), stop=(p == NP - 1), perf_mode=DR)
        cx.ps[i][q] = ps
        _emit_drain_q(nc, cx, i, q)


def _emit_mm_pairstream(nc, cx, i):
    """Tile i matmuls pair-major: consumes W pairs as they are quantized
    during the W-load phase (holds its 4 psum banks until the last pair)."""
    xqT = cx.xqT[i]
    cx.ps[i] = {q: cx.mmp.tile([128, QW], F32, tag="mm", name=f"ps{i}_{q}")
                for q in range(4)}
    for n, p in enumerate(PAIR_ORDER):
        for q in range(4):
            nc.tensor.matmul(cx.ps[i][q][:], xqT[:, 2 * p:2 * p + 2, :],
                             cx.wcT[p][:, :, q * QW:(q + 1) * QW],
                             start=(n == 0), stop=(n == NP - 1), perf_mode=DR)
    for q in range(4):
        _emit_drain_q(nc, cx, i, q)


def build():
    nc = bacc.Bacc("TRN2", target_bir_lowering=False, debug=False,
                   num_devices=NCORES)
    cx = Ctx()
    cx.x_d = nc.dram_tensor("x", [T, DIN], F32, kind="ExternalInput")
    cx.wT_d = nc.dram_tensor("wT", [DIN, DOUT], F32, kind="ExternalInput")
    cx.out_d = nc.dram_tensor("out", [T, DOUT], BF16, kind="ExternalOutput")
    cx.xqT, cx.negos, cx.b, cx.S = {}, {}, {}, {}
    cx.ps, cx.xf, cx.ob = {}, {}, {}

    with tile.TileContext(nc) as tc:
        with (
            tc.tile_pool(name="singles", bufs=1) as singles,
            tc.tile_pool(name="wf", bufs=10) as wfp,
            tc.tile_pool(name="x", bufs=3) as xp,
            tc.tile_pool(name="y", bufs=2) as yp,
            tc.tile_pool(name="xq", bufs=2) as xqp,
            tc.tile_pool(name="xqT", bufs=3) as xqTp,
            tc.tile_pool(name="dsc", bufs=1) as dscp,
            tc.tile_pool(name="st", bufs=16) as st,
            tc.tile_pool(name="outp", bufs=6) as outp,
            tc.tile_pool(name="mmps", bufs=4, space="PSUM") as mmp,
            tc.tile_pool(name="tpps", bufs=1, space="PSUM") as tpp,
            tc.tile_pool(name="grps", bufs=2, space="PSUM") as grp,
        ):
            cx.xp, cx.yp, cx.xqp, cx.xqTp = xp, yp, xqp, xqTp
            cx.st, cx.outp, cx.dscp = st, outp, dscp
            cx.mmp, cx.tpp, cx.grp = mmp, tpp, grp

            # ---- preamble
            dummy = singles.tile([128, 1], F32)
            nc.vector.memset(dummy[:], 1.0)
            dummy2 = singles.tile([128, 1], F32)
            for fn in (AF.Sqrt, AF.Identity, AF.Copy):
                nc.scalar.activation(out=dummy2[:], in_=dummy[:], func=fn)
            cx.idn = singles.tile([128, 128], BF16)
            make_identity(nc, cx.idn[:])
            cx.c_col = singles.tile([128, 1], F32)
            nc.vector.memset(cx.c_col[:], C_MAGIC)

            # ---- SP DMA queue order: w0, w1 (gamma chunks first), x0
            wf = {}
            for j in range(2):
                wf[j] = wfp.tile([128, DOUT], F32, tag="wf", name=f"w_{j}")
                nc.sync.dma_start(wf[j][:],
                                  cx.wT_d.ap()[j * 128:(j + 1) * 128, :])
            cx.xf[0] = xp.tile([128, DIN], F32, tag="xf", name="xf0")
            nc.sync.dma_start(cx.xf[0][:], cx.x_d.ap()[0:128, :])

            # ---- gamma accumulation passes on ACT
            wsum = singles.tile([128, 2], F32)
            sc0 = yp.tile([128, DOUT], F32, tag="y", name="wabs_s0")
            nc.scalar.activation(out=sc0[:], in_=wf[0][:], func=AF.Identity,
                                 accum_out=wsum[:, 0:1])
            sc1 = yp.tile([128, DOUT], F32, tag="y", name="wabs_s1")
            nc.scalar.activation(out=sc1[:], in_=wf[1][:], func=AF.Identity,
                                 accum_out=wsum[:, 1:2])

            # ---- tile 0 chain (heads every engine queue)
            _emit_x_chain(nc, cx, 0)

            # ---- collective: AllGather of 8 local-slice sums
            wsum1 = singles.tile([128, 1], F32)
            nc.vector.tensor_reduce(out=wsum1[:], in_=wsum[:],
                                    axis=mybir.AxisListType.X, op=ALU.add)
            total_loc = singles.tile([128, 1], F32)
            nc.gpsimd.partition_all_reduce(total_loc[:], wsum1[:],
                                           channels=128,
                                           reduce_op=bass_isa.ReduceOp.add)
            cc_in = singles.tile([1, 1], F32, space="DRAM")
            cc_out = singles.tile([1, NCORES], F32, space="DRAM")
            cx.ccst_inst = nc.gpsimd.dma_start(cc_in[:], total_loc[0:1, 0:1])
            nc.gpsimd.collective_compute(
                "AllGather", ALU.bypass,
                replica_groups=[list(range(NCORES))],
                ins=[cc_in[:]], outs=[cc_out[:]])
            gsum = singles.tile([1, NCORES], F32)
            cx.ccld_inst = nc.gpsimd.dma_start(gsum[:], cc_out[:])
            gtot = singles.tile([1, 1], F32)
            nc.vector.tensor_reduce(out=gtot[:], in_=gsum[:],
                                    axis=mybir.AxisListType.X, op=ALU.add)
            thr0 = singles.tile([1, 1], F32)
            nc.gpsimd.tensor_scalar(out=thr0[:], in0=gtot[:],
                                    scalar1=0.5 / (DIN * DOUT),
                                    scalar2=0.5 * EPS_GAMMA,
                                    op0=ALU.mult, op1=ALU.add)
            thr = singles.tile([128, 1], F32)
            nc.gpsimd.partition_broadcast(thr[:], thr0[:])

            # ---- W loads w2..w15 (SP queue).  Bulk loads yield the DMA
            # pool to the tiny collective bounce transfers via dep edges.
            from concourse.tile_rust import add_dep_helper
            for j in range(2, KC):
                wf[j] = wfp.tile([128, DOUT], F32, tag="wf", name=f"w_{j}")
                ld = nc.sync.dma_start(wf[j][:],
                                       cx.wT_d.ap()[j * 128:(j + 1) * 128, :])
                if j >= 4:
                    add_dep_helper(ld.ins, cx.ccst_inst.ins, sync=True,
                                   reason="yield DMA pool to cc_in store")
                if j >= 9:
                    add_dep_helper(ld.ins, cx.ccld_inst.ins, sync=True,
                                   reason="yield DMA pool to cc_out load")

            # ---- W quantization: wc = (w <= thr) -> fp8
            cx.wcT = {p: singles.tile([128, 2, DOUT], FP8, name=f"wcT{p}")
                      for p in range(NP)}

            def emit_quant(j):
                eng = {"dve": nc.vector, "pool": nc.gpsimd}[QUANT_ENG[j]]
                eng.tensor_scalar(out=cx.wcT[j // 2][:, j % 2, :],
                                  in0=wf[j][:], scalar1=thr[:], scalar2=None,
                                  op0=ALU.is_le)

            for j in range(KC):
                if QUANT_ENG[j] == "dve":
                    emit_quant(j)
            for j in range(KC):
                if QUANT_ENG[j] == "pool":
                    emit_quant(j)

            # ---- tile 0 matmuls pair-streamed during the W phase
            cx.ob[0] = outp.tile([128, DOUT], BF16, tag="ob", name="ob0")
            _emit_mm_pairstream(nc, cx, 0)

            # ---- x1..x15 + crunch
            stores_emitted = 0

            def emit_store(i):
                nc.sync.dma_start(cx.out_d.ap()[i * 128:(i + 1) * 128, :],
                                  cx.ob[i][:])

            for i in range(1, TP):
                cx.xf[i] = xp.tile([128, DIN], F32, tag="xf", name=f"xf{i}")
                nc.sync.dma_start(cx.xf[i][:],
                                  cx.x_d.ap()[i * 128:(i + 1) * 128, :])
                if i >= ST_INTERLEAVE_AT:
                    for _ in range(2):
                        if stores_emitted < i - 3:
                            emit_store(stores_emitted)
                            stores_emitted += 1
                _emit_x_chain(nc, cx, i)
                cx.ob[i] = outp.tile([128, DOUT], BF16, tag="ob",
                                     name=f"ob{i}")
                if i >= 2:
                    _emit_mm_burst(nc, cx, i - 1)
            _emit_mm_burst(nc, cx, TP - 1)
            while stores_emitted < TP:
                emit_store(stores_emitted)
                stores_emitted += 1

    nc.compile()
    return nc


_NC_CACHE = []


def kernel(x: np.ndarray, weight: np.ndarray) -> np.ndarray:
    assert x.shape == (B, S, DIN) and weight.shape == (DOUT, DIN)
    if not _NC_CACHE:
        _NC_CACHE.append(build())
    nc = _NC_CACHE[0]

    xs = np.ascontiguousarray(x.reshape(B * S, DIN), dtype=np.float32)
    wT = np.ascontiguousarray(weight.T.astype(np.float32))
    in_maps = []
    for k in range(NCORES):
        r = 256 * k
        # roll the contraction dim on both operands so chunks 0,1 of this
        # core's wT stream are its private gamma slice (no separate wg load)
        wk = np.ascontiguousarray(np.roll(wT, -r, axis=0))
        xk = np.ascontiguousarray(np.roll(xs[k * T:(k + 1) * T], -r, axis=1))
        in_maps.append({"x": xk, "wT": wk})
    res = run_bass_kernel_spmd(nc, in_maps, core_ids=list(range(NCORES)))
    out = np.concatenate(
        [np.asarray(res.results[k]["out"]).astype(np.float32)
         for k in range(NCORES)], axis=0)
    return np.ascontiguousarray(out.reshape(B, S, DOUT))
